# revision 2
# baseline (speedup 1.0000x reference)
"""ClockworkRNN Trainium2 kernel (Bass/Tile), data-parallel over batch on 8 cores.

Reference semantics (see problem):
  x = X @ W + b                      # (B, T, 512)
  per step t: group i (of 8, 64 units each, period 2^i) updates iff t % 2^i == 0
    upd_i = x[t, i*64:(i+1)*64] + h[:, i*64:] @ Wc_i
    h     = tanh(concat(where(update, upd_i, h_i)))    # tanh applied to ALL units
  return h after t = T-1             # (B, 512)

Active groups at step t are always a prefix 0..g, g = min(ntz(t), 7) (g=7 at t=0).

Device design (per core, B_LOC=8 batch rows):
  - State hT kept transposed in SBUF as fp16: tile (128 part = unit within
    chunk, 4 chunks of 128 units, 8 batch).
  - X is bulk-transposed on the PE (128x128 fp32 transposes via identity) into
    streaming fp16 SBUF tiles xt (d, t, b) via DVE copies; a bulk "Phase-A"
    projection computes xq = W.T @ xt (+b) per 128-step block with fp16
    matmuls split into N=128 pieces (so scan-critical matmuls never stall
    long behind them), results copied PSUM->SBUF on the DVE (not the
    scan-critical Activation engine). Both are pipelined 2 blocks ahead of the scan.
  - Per step: one PSUM bank tile (128, 4, 8). For each updated chunk, ONE
    identity-inject matmul (lhsT = I, or I with zeroed upper cols for
    pass-through chunks) moves x into PSUM (start=True clears has_written on
    all 128 partitions); recurrence matmuls accumulate on top using
    host-packed 128x128 fp16 weight tiles where the inactive upper half-chunk
    of an even-g step carries an identity block, so tanh(PSUM) reproduces
    tanh(h_old) for non-updated units within the same ACT instruction.
    Matmuls that depend on the freshest h chunk (contraction c=0) are emitted
    LAST so the critical-path tail after the previous step's tanh is minimal.
  - ACT: instr B = tanh(hT_prev[suffix chunks]) -> hT (off critical path);
    instr A = tanh(PSUM[0:mh+1 chunks]) -> hT (critical path).
"""

import numpy as np

import concourse.bacc as bacc
import concourse.mybir as mybir
import concourse.tile as tile
from concourse.bass_utils import run_bass_kernel_spmd

# ---- problem constants (hardcoded per harness contract) ----
N_CORES = 8
B_FULL = 64
B_LOC = B_FULL // N_CORES  # 8
T_FULL = 2048
D_IN = 256
D_OUT = 512
BLOCK = 128  # scan steps per t-block
FP32 = mybir.dt.float32
FP16 = mybir.dt.float16
TANH = mybir.ActivationFunctionType.Tanh
COPY = mybir.ActivationFunctionType.Copy


def _g_of(t: int) -> int:
    if t == 0:
        return 7
    return min((t & -t).bit_length() - 1, 7)


def pack_rec_weights(Wcs: list[np.ndarray]) -> tuple[np.ndarray, dict]:
    """Pack recurrence weights into (20, 128, 128) fp32 lhsT tiles.

    Tile (m, v, c): lhsT for PSUM out-chunk m (units 128m..128m+128),
    contraction K-chunk c (h units 128c..128c+128), variant v
    (1 = upper group 2m+1 active, 0 = pass-through identity).
    cols 0..63   -> group 2m   (always active when chunk m is touched)
    cols 64..127 -> group 2m+1 (Wc if active, identity block if pass)
    """
    tiles = []
    index = {}
    for m in range(4):
        for v in (0, 1):
            for c in range(m, 4):
                w = np.zeros((128, 128), dtype=np.float32)
                a = 2 * m
                bgrp = 2 * m + 1
                for kk in range(128):
                    k = 128 * c + kk  # global h unit index
                    if k >= 64 * a:
                        w[kk, 0:64] = Wcs[a][k - 64 * a, :]
                    if v == 1:
                        if k >= 64 * bgrp:
                            w[kk, 64:128] = Wcs[bgrp][k - 64 * bgrp, :]
                    elif c == m and kk >= 64:
                        w[kk, kk] = 1.0
                index[(m, v, c)] = len(tiles)
                tiles.append(w)
    return np.stack(tiles), index


_REC_INDEX = pack_rec_weights(
    [np.zeros(((8 - i) * 64, 64), np.float32) for i in range(8)]
)[1]


def build_program(T: int, b_nonzero: bool = False, reps: int = 1):
    """Emit the full SPMD program; returns compiled nc.

    reps > 1 replays the whole computation that many times back-to-back
    (identical device work per rep) — used by test.py to measure device
    execution time as a wall-clock slope, cancelling transfer overheads.
    """
    assert T % BLOCK == 0
    n_blk = T // BLOCK
    HB = BLOCK // 2
    QB = BLOCK // 8  # phase-A matmul N = QB * B_LOC = 128 per piece
    nc = bacc.Bacc(
        "TRN2", target_bir_lowering=False, debug=False, num_devices=N_CORES
    )

    X_ap = nc.dram_tensor("X", [B_LOC, T, D_IN], FP32, kind="ExternalInput").ap()
    W_ap = nc.dram_tensor("W", [D_IN, D_OUT], FP16, kind="ExternalInput").ap()
    RW_ap = nc.dram_tensor("RW", [20, 128, 128], FP16, kind="ExternalInput").ap()
    # ID2[0] = I_128; ID2[1] = I with cols 64..127 zeroed (pass-through inject)
    # fp16 copies for inject matmuls + one fp32 identity for PE transposes.
    ID2_ap = nc.dram_tensor("ID2", [2, 128, 128], FP16, kind="ExternalInput").ap()
    IDT_ap = nc.dram_tensor("IDT", [128, 128], FP32, kind="ExternalInput").ap()
    if b_nonzero:
        BV_ap = nc.dram_tensor("BV", [128, 4], FP32, kind="ExternalInput").ap()
    out_ap = nc.dram_tensor("out", [128, 4, B_LOC], FP32, kind="ExternalOutput").ap()

    with tile.TileContext(nc) as tc:
        with (
            tc.tile_pool(name="const", bufs=1) as constp,
            tc.tile_pool(name="xraw", bufs=6) as xrawp,
            tc.tile_pool(name="xt0", bufs=3) as xt0p,
            tc.tile_pool(name="xt1", bufs=3) as xt1p,
            tc.tile_pool(name="xq", bufs=3) as xqp,
            tc.tile_pool(name="hp", bufs=6) as hp,
            tc.tile_pool(name="ps", bufs=5, space="PSUM") as psp,
            tc.tile_pool(name="pstr", bufs=1, space="PSUM") as pstrp,
            tc.tile_pool(name="psx", bufs=2, space="PSUM") as psxp,
        ):
            # ---- persistent weights ----
            w_sb = constp.tile([128, 2, D_OUT], FP16, tag="w_sb", name="w_sb")
            nc.sync.dma_start(w_sb[:], W_ap.rearrange("(c p) u -> p c u", p=128))
            rw_sb = constp.tile([128, 20, 128], FP16, tag="rw_sb", name="rw_sb")
            nc.sync.dma_start(rw_sb[:], RW_ap.rearrange("n k m -> k n m"))
            id2_sb = constp.tile([128, 2, 128], FP16, tag="id2_sb", name="id2_sb")
            nc.sync.dma_start(id2_sb[:], ID2_ap.rearrange("v k m -> k v m"))
            idt_sb = constp.tile([128, 128], FP32, tag="idt_sb", name="idt_sb")
            nc.sync.dma_start(idt_sb[:], IDT_ap)
            if b_nonzero:
                bv_sb = constp.tile([128, 4], FP32, tag="bv_sb", name="bv_sb")
                nc.sync.dma_start(bv_sb[:], BV_ap)

            xt_blocks: dict = {}
            xq_blocks: dict = {}
            xraw_tiles: dict = {}

            def emit_xdma(blk, bb):
                xr = xrawp.tile([128, D_IN], FP32, tag="xraw", name="xr")
                nc.sync.dma_start(
                    xr[:], X_ap[bb, blk * BLOCK : (blk + 1) * BLOCK, :]
                )
                xraw_tiles[(blk, bb)] = xr

            def emit_transpose(blk, pair):
                bb, dc = pair // 2, pair % 2
                if pair == 0:
                    xt_blocks[blk] = [
                        xt0p.tile([128, BLOCK, B_LOC], FP16, tag="xt0", name="xt0"),
                        xt1p.tile([128, BLOCK, B_LOC], FP16, tag="xt1", name="xt1"),
                    ]
                xr = xraw_tiles[(blk, bb)]
                ptr = pstrp.tile([128, 128], FP32, tag="pstr", name="ptr")
                nc.tensor.transpose(
                    ptr[:], xr[:, dc * 128 : (dc + 1) * 128], idt_sb[:]
                )
                nc.vector.tensor_copy(xt_blocks[blk][dc][:, :, bb], ptr[:])
                if pair == 15:
                    for bx in range(8):
                        del xraw_tiles[(blk, bx)]

            def emit_phase_a(blk, unit):
                # unit in 0..7 -> (m, half): 8 matmuls (N=128) + 1 DVE copy
                m, half = unit // 2, unit % 2
                if unit == 0:
                    xq_blocks[blk] = [
                        xqp.tile([128, BLOCK, B_LOC], FP16, tag=f"xq{m2}", name="xq")
                        for m2 in range(4)
                    ]
                xt = xt_blocks[blk]
                px = psxp.tile([128, HB * B_LOC], FP32, tag="psx", name="px")
                for q in range(4):
                    t0 = half * HB + q * QB
                    for dc in range(2):
                        nc.tensor.matmul(
                            px[:, q * QB * B_LOC : (q + 1) * QB * B_LOC],
                            w_sb[:, dc, 128 * m : 128 * m + 128],
                            xt[dc][:, t0 : t0 + QB, :],
                            start=dc == 0,
                            stop=dc == 1,
                        )
                dst = xq_blocks[blk][m][:, half * HB : (half + 1) * HB, :]
                if b_nonzero:
                    nc.scalar.activation(dst, px[:], COPY, bias=bv_sb[:, m : m + 1])
                else:
                    nc.vector.tensor_copy(dst, px[:])
                if unit == 7:
                    del xt_blocks[blk]

            def emit_step(t, h_prev):
                g = _g_of(t)
                mh = g // 2
                ps_t = psp.tile([128, 4, B_LOC], FP32, tag="ps", name="ps")
                h_t = hp.tile([128, 4, B_LOC], FP16, tag="h", name="h")
                xq = xq_blocks[t // BLOCK]
                t_off = t % BLOCK
                # --- x inject matmuls (identity lhsT; zeroed upper half for
                # pass-through chunks). start=True on chunk 0 clears the bank.
                for m in range(mh + 1):
                    pass_chunk = g < 2 * m + 1
                    nc.tensor.matmul(
                        ps_t[:, m, :],
                        id2_sb[:, 1 if pass_chunk else 0, :],
                        xq[m][:, t_off, :],
                        start=m == 0,
                        stop=(t == 0 and m == mh),
                    )
                # --- off-critical-path tanh of untouched suffix chunks ---
                if mh < 3:
                    nc.scalar.activation(
                        h_t[:, mh + 1 : 4, :], h_prev[:, mh + 1 : 4, :], TANH
                    )
                # --- recurrence matmuls; c=0 (freshest h chunk) last ---
                if t > 0:
                    for c in range(3, -1, -1):
                        for m in range(min(mh, c) + 1):
                            v = 1 if g >= 2 * m + 1 else 0
                            nc.tensor.matmul(
                                ps_t[:, m, :],
                                rw_sb[:, _REC_INDEX[(m, v, c)], :],
                                h_prev[:, c, :],
                                start=False,
                                stop=(c, m) == (0, mh),
                            )
                # --- critical-path tanh of updated prefix ---
                nc.scalar.activation(
                    h_t[:, 0 : mh + 1, :], ps_t[:, 0 : mh + 1, :], TANH
                )
                return h_t

            for _rep in range(reps):
                # prologue: prepare blocks 0 (and 1) fully
                for j in range(min(2, n_blk)):
                    for bb in range(8):
                        emit_xdma(j, bb)
                    for pair in range(16):
                        emit_transpose(j, pair)
                    for unit in range(8):
                        emit_phase_a(j, unit)

                h_prev = None
                for blk in range(n_blk):
                    for s in range(BLOCK):
                        t = blk * BLOCK + s
                        if blk + 2 < n_blk:
                            if s < 8:
                                emit_xdma(blk + 2, s)
                            if s % 8 == 4:
                                emit_transpose(blk + 2, s // 8)
                        if blk + 1 < n_blk and blk > 0:
                            if s % 16 == 12:
                                emit_phase_a(blk + 1, s // 16)
                        h_prev = emit_step(t, h_prev)
                    if blk - 1 in xq_blocks:
                        del xq_blocks[blk - 1]
                    if blk in xq_blocks and blk == n_blk - 1:
                        pass
                # final cast fp16 -> fp32 for the output DMA
                out_sb = hp.tile([128, 4, B_LOC], FP32, tag="out_sb", name="out_sb")
                nc.vector.tensor_copy(out_sb[:], h_prev[:])
                nc.sync.dma_start(out_ap, out_sb[:])
                xq_blocks.clear()

    nc.compile()
    return nc


# ---- host-side entry point ----
_PROG_CACHE: dict = {}


def _get_prog(T: int, b_nonzero: bool, reps: int = 1):
    key = (T, b_nonzero, reps)
    if key not in _PROG_CACHE:
        _PROG_CACHE[key] = build_program(T, b_nonzero=b_nonzero, reps=reps)
    return _PROG_CACHE[key]


def make_in_maps(X, W, b, Wcs, b_nonzero: bool):
    X = np.ascontiguousarray(np.asarray(X, dtype=np.float32))
    W = np.ascontiguousarray(np.asarray(W, dtype=np.float16))
    b = np.asarray(b, dtype=np.float32)
    rec_w, _ = pack_rec_weights([np.asarray(w, dtype=np.float32) for w in Wcs])
    rec_w = rec_w.astype(np.float16)
    id2 = np.stack([np.eye(128, dtype=np.float16)] * 2)
    id2[1, :, 64:] = 0.0
    idt = np.eye(128, dtype=np.float32)
    in_maps = []
    for c in range(N_CORES):
        m = {
            "X": X[c * B_LOC : (c + 1) * B_LOC],
            "W": W,
            "RW": rec_w,
            "ID2": id2,
            "IDT": idt,
        }
        if b_nonzero:
            m["BV"] = np.ascontiguousarray(b.reshape(4, 128).T)
        in_maps.append(m)
    return in_maps


def gather(results) -> np.ndarray:
    out = np.empty((B_FULL, D_OUT), dtype=np.float32)
    for c in range(N_CORES):
        o = results[c]["out"]  # (128, 4, B_LOC): unit = 128*chunk + partition
        out[c * B_LOC : (c + 1) * B_LOC] = o.transpose(2, 1, 0).reshape(B_LOC, D_OUT)
    return out


def kernel(X, W, b, Wc0, Wc1, Wc2, Wc3, Wc4, Wc5, Wc6, Wc7) -> np.ndarray:
    Wcs = [Wc0, Wc1, Wc2, Wc3, Wc4, Wc5, Wc6, Wc7]
    b_np = np.asarray(b, dtype=np.float32)
    b_nonzero = bool(np.any(b_np != 0))
    T = int(np.asarray(X).shape[1])
    nc = _get_prog(T, b_nonzero)
    in_maps = make_in_maps(X, W, b_np, Wcs, b_nonzero)
    res = run_bass_kernel_spmd(nc, in_maps, core_ids=list(range(N_CORES)))
    return gather(res.results)


# revision 3
# speedup vs baseline: 16.1584x; 16.1584x over previous
"""ClockworkRNN Trainium2 kernel (Bass/Tile), data-parallel over batch on 8 cores.

Reference semantics (see problem):
  x = X @ W + b                      # (B, T, 512)
  per step t: group i (of 8, 64 units each, period 2^i) updates iff t % 2^i == 0
    upd_i = x[t, i*64:(i+1)*64] + h[:, i*64:] @ Wc_i
    h     = tanh(concat(where(update, upd_i, h_i)))    # tanh applied to ALL units
  return h after t = T-1             # (B, 512)

Active groups at step t are always a prefix 0..g, g = min(ntz(t), 7) (g=7 at t=0).

Device design (per core, B_LOC=8 batch rows):
  - State hT kept transposed in SBUF as fp16: tile (128 part = unit within
    chunk, 4 chunks of 128 units, 8 batch).
  - X is bulk-transposed on the PE (128x128 fp32 transposes via identity) into
    streaming fp16 SBUF tiles xt (d, t, b) via DVE copies; a bulk "Phase-A"
    projection computes xq = W.T @ xt (+b) per 128-step block with fp16
    matmuls split into N=128 pieces (so scan-critical matmuls never stall
    long behind them), results copied PSUM->SBUF on the DVE (not the
    scan-critical Activation engine). Both are pipelined 2 blocks ahead of the scan.
  - Per step: one PSUM bank tile (128, 4, 8). For each updated chunk, ONE
    identity-inject matmul (lhsT = I, or I with zeroed upper cols for
    pass-through chunks) moves x into PSUM (start=True clears has_written on
    all 128 partitions); recurrence matmuls accumulate on top using
    host-packed 128x128 fp16 weight tiles where the inactive upper half-chunk
    of an even-g step carries an identity block, so tanh(PSUM) reproduces
    tanh(h_old) for non-updated units within the same ACT instruction.
    Matmuls that depend on the freshest h chunk (contraction c=0) are emitted
    LAST so the critical-path tail after the previous step's tanh is minimal.
  - ACT: instr B = tanh(hT_prev[suffix chunks]) -> hT (off critical path);
    instr A = tanh(PSUM[0:mh+1 chunks]) -> hT (critical path).
"""

import numpy as np

import concourse.bacc as bacc
import concourse.mybir as mybir
import concourse.tile as tile
from concourse.bass_utils import run_bass_kernel_spmd

# ---- problem constants (hardcoded per harness contract) ----
N_CORES = 8
B_FULL = 64
B_LOC = B_FULL // N_CORES  # 8
T_FULL = 2048
D_IN = 256
D_OUT = 512
BLOCK = 128  # scan steps per t-block
FP32 = mybir.dt.float32
FP16 = mybir.dt.float16
TANH = mybir.ActivationFunctionType.Tanh
COPY = mybir.ActivationFunctionType.Copy


def _g_of(t: int) -> int:
    if t == 0:
        return 7
    return min((t & -t).bit_length() - 1, 7)


def pack_rec_weights(Wcs: list[np.ndarray]) -> tuple[np.ndarray, dict]:
    """Pack recurrence weights into (20, 128, 128) fp32 lhsT tiles.

    Tile (m, v, c): lhsT for PSUM out-chunk m (units 128m..128m+128),
    contraction K-chunk c (h units 128c..128c+128), variant v
    (1 = upper group 2m+1 active, 0 = pass-through identity).
    cols 0..63   -> group 2m   (always active when chunk m is touched)
    cols 64..127 -> group 2m+1 (Wc if active, identity block if pass)
    """
    tiles = []
    index = {}
    for m in range(4):
        for v in (0, 1):
            for c in range(m, 4):
                w = np.zeros((128, 128), dtype=np.float32)
                a = 2 * m
                bgrp = 2 * m + 1
                for kk in range(128):
                    k = 128 * c + kk  # global h unit index
                    if k >= 64 * a:
                        w[kk, 0:64] = Wcs[a][k - 64 * a, :]
                    if v == 1:
                        if k >= 64 * bgrp:
                            w[kk, 64:128] = Wcs[bgrp][k - 64 * bgrp, :]
                    elif c == m and kk >= 64:
                        w[kk, kk] = 1.0
                index[(m, v, c)] = len(tiles)
                tiles.append(w)
    return np.stack(tiles), index


_REC_INDEX = pack_rec_weights(
    [np.zeros(((8 - i) * 64, 64), np.float32) for i in range(8)]
)[1]


def build_program(T: int, b_nonzero: bool = False, reps: int = 1):
    """Emit the full SPMD program; returns compiled nc.

    reps > 1 replays the whole computation that many times back-to-back
    (identical device work per rep) — used by test.py to measure device
    execution time as a wall-clock slope, cancelling transfer overheads.
    """
    assert T % BLOCK == 0
    n_blk = T // BLOCK
    HB = BLOCK // 2
    QB = BLOCK // 8  # phase-A matmul N = QB * B_LOC = 128 per piece
    nc = bacc.Bacc(
        "TRN2", target_bir_lowering=False, debug=False, num_devices=N_CORES
    )

    X_ap = nc.dram_tensor("X", [B_LOC, T, D_IN], FP32, kind="ExternalInput").ap()
    W_ap = nc.dram_tensor("W", [D_IN, D_OUT], FP16, kind="ExternalInput").ap()
    RW_ap = nc.dram_tensor("RW", [20, 128, 128], FP16, kind="ExternalInput").ap()
    # ID2[0] = I_128; ID2[1] = I with cols 64..127 zeroed (pass-through inject)
    # fp16 copies for inject matmuls + one fp32 identity for PE transposes.
    ID2_ap = nc.dram_tensor("ID2", [2, 128, 128], FP16, kind="ExternalInput").ap()
    IDT_ap = nc.dram_tensor("IDT", [128, 128], FP32, kind="ExternalInput").ap()
    if b_nonzero:
        BV_ap = nc.dram_tensor("BV", [128, 4], FP32, kind="ExternalInput").ap()
    out_ap = nc.dram_tensor("out", [128, 4, B_LOC], FP32, kind="ExternalOutput").ap()

    with tile.TileContext(nc) as tc:
        with (
            tc.tile_pool(name="const", bufs=1) as constp,
            tc.tile_pool(name="xraw", bufs=6) as xrawp,
            tc.tile_pool(name="xt0", bufs=3) as xt0p,
            tc.tile_pool(name="xt1", bufs=3) as xt1p,
            tc.tile_pool(name="xq", bufs=3) as xqp,
            tc.tile_pool(name="hp", bufs=6) as hp,
            tc.tile_pool(name="ps", bufs=5, space="PSUM") as psp,
            tc.tile_pool(name="pstr", bufs=1, space="PSUM") as pstrp,
            tc.tile_pool(name="psx", bufs=2, space="PSUM") as psxp,
        ):
            # ---- persistent weights ----
            w_sb = constp.tile([128, 2, D_OUT], FP16, tag="w_sb", name="w_sb")
            nc.sync.dma_start(w_sb[:], W_ap.rearrange("(c p) u -> p c u", p=128))
            rw_sb = constp.tile([128, 20, 128], FP16, tag="rw_sb", name="rw_sb")
            nc.sync.dma_start(rw_sb[:], RW_ap.rearrange("n k m -> k n m"))
            id2_sb = constp.tile([128, 2, 128], FP16, tag="id2_sb", name="id2_sb")
            nc.sync.dma_start(id2_sb[:], ID2_ap.rearrange("v k m -> k v m"))
            idt_sb = constp.tile([128, 128], FP32, tag="idt_sb", name="idt_sb")
            nc.sync.dma_start(idt_sb[:], IDT_ap)
            if b_nonzero:
                bv_sb = constp.tile([128, 4], FP32, tag="bv_sb", name="bv_sb")
                nc.sync.dma_start(bv_sb[:], BV_ap)

            xt_blocks: dict = {}
            xq_blocks: dict = {}
            xraw_tiles: dict = {}

            def emit_xdma(blk, bb):
                xr = xrawp.tile([128, D_IN], FP32, tag="xraw", name="xr")
                nc.sync.dma_start(
                    xr[:], X_ap[bb, blk * BLOCK : (blk + 1) * BLOCK, :]
                )
                xraw_tiles[(blk, bb)] = xr

            def emit_transpose(blk, pair):
                bb, dc = pair // 2, pair % 2
                if pair == 0:
                    xt_blocks[blk] = [
                        xt0p.tile([128, BLOCK, B_LOC], FP16, tag="xt0", name="xt0"),
                        xt1p.tile([128, BLOCK, B_LOC], FP16, tag="xt1", name="xt1"),
                    ]
                xr = xraw_tiles[(blk, bb)]
                ptr = pstrp.tile([128, 128], FP32, tag="pstr", name="ptr")
                nc.tensor.transpose(
                    ptr[:], xr[:, dc * 128 : (dc + 1) * 128], idt_sb[:]
                )
                nc.vector.tensor_copy(xt_blocks[blk][dc][:, :, bb], ptr[:])
                if pair == 15:
                    for bx in range(8):
                        del xraw_tiles[(blk, bx)]

            def emit_phase_a(blk, unit):
                # unit in 0..7 -> (m, half): 8 matmuls (N=128) + 1 DVE copy
                m, half = unit // 2, unit % 2
                if unit == 0:
                    xq_blocks[blk] = [
                        xqp.tile([128, BLOCK, B_LOC], FP16, tag=f"xq{m2}", name="xq")
                        for m2 in range(4)
                    ]
                xt = xt_blocks[blk]
                px = psxp.tile([128, HB * B_LOC], FP32, tag="psx", name="px")
                for q in range(4):
                    t0 = half * HB + q * QB
                    for dc in range(2):
                        nc.tensor.matmul(
                            px[:, q * QB * B_LOC : (q + 1) * QB * B_LOC],
                            w_sb[:, dc, 128 * m : 128 * m + 128],
                            xt[dc][:, t0 : t0 + QB, :],
                            start=dc == 0,
                            stop=dc == 1,
                        )
                dst = xq_blocks[blk][m][:, half * HB : (half + 1) * HB, :]
                if b_nonzero:
                    nc.vector.tensor_scalar_add(dst, px[:], bv_sb[:, m : m + 1])
                else:
                    nc.vector.tensor_copy(dst, px[:])
                if unit == 7:
                    del xt_blocks[blk]

            def emit_step(t, h_prev):
                g = _g_of(t)
                mh = g // 2
                ps_t = psp.tile([128, 4, B_LOC], FP32, tag="ps", name="ps")
                h_t = hp.tile([128, 4, B_LOC], FP16, tag="h", name="h")
                xq = xq_blocks[t // BLOCK]
                t_off = t % BLOCK
                # --- x inject matmuls (identity lhsT; zeroed upper half for
                # pass-through chunks). start=True on chunk 0 clears the bank.
                for m in range(mh + 1):
                    pass_chunk = g < 2 * m + 1
                    nc.tensor.matmul(
                        ps_t[:, m, :],
                        id2_sb[:, 1 if pass_chunk else 0, :],
                        xq[m][:, t_off, :],
                        start=m == 0,
                        stop=(t == 0 and m == mh),
                    )
                # --- off-critical-path tanh of untouched suffix chunks ---
                if mh < 3:
                    nc.scalar.activation(
                        h_t[:, mh + 1 : 4, :], h_prev[:, mh + 1 : 4, :], TANH
                    )
                # --- recurrence matmuls; c=0 (freshest h chunk) last ---
                if t > 0:
                    for c in range(3, -1, -1):
                        for m in range(min(mh, c) + 1):
                            v = 1 if g >= 2 * m + 1 else 0
                            nc.tensor.matmul(
                                ps_t[:, m, :],
                                rw_sb[:, _REC_INDEX[(m, v, c)], :],
                                h_prev[:, c, :],
                                start=False,
                                stop=(c, m) == (0, mh),
                            )
                # --- critical-path tanh of updated prefix ---
                nc.scalar.activation(
                    h_t[:, 0 : mh + 1, :], ps_t[:, 0 : mh + 1, :], TANH
                )
                return h_t

            for _rep in range(reps):
                # prologue: prepare blocks 0 (and 1) fully
                for j in range(min(2, n_blk)):
                    for bb in range(8):
                        emit_xdma(j, bb)
                    for pair in range(16):
                        emit_transpose(j, pair)
                    for unit in range(8):
                        emit_phase_a(j, unit)

                h_prev = None
                for blk in range(n_blk):
                    for s in range(BLOCK):
                        t = blk * BLOCK + s
                        if blk + 2 < n_blk:
                            if s < 8:
                                emit_xdma(blk + 2, s)
                            if s % 8 == 4:
                                emit_transpose(blk + 2, s // 8)
                        if blk + 1 < n_blk and blk > 0:
                            if s % 16 == 12:
                                emit_phase_a(blk + 1, s // 16)
                        h_prev = emit_step(t, h_prev)
                    if blk - 1 in xq_blocks:
                        del xq_blocks[blk - 1]
                    if blk in xq_blocks and blk == n_blk - 1:
                        pass
                # final cast fp16 -> fp32 for the output DMA
                out_sb = hp.tile([128, 4, B_LOC], FP32, tag="out_sb", name="out_sb")
                nc.vector.tensor_copy(out_sb[:], h_prev[:])
                nc.sync.dma_start(out_ap, out_sb[:])
                xq_blocks.clear()

    nc.compile()
    return nc


# ---- host-side entry point ----
_PROG_CACHE: dict = {}


def _get_prog(T: int, b_nonzero: bool, reps: int = 1):
    key = (T, b_nonzero, reps)
    if key not in _PROG_CACHE:
        _PROG_CACHE[key] = build_program(T, b_nonzero=b_nonzero, reps=reps)
    return _PROG_CACHE[key]


def make_in_maps(X, W, b, Wcs, b_nonzero: bool):
    X = np.ascontiguousarray(np.asarray(X, dtype=np.float32))
    W = np.ascontiguousarray(np.asarray(W, dtype=np.float16))
    b = np.asarray(b, dtype=np.float32)
    rec_w, _ = pack_rec_weights([np.asarray(w, dtype=np.float32) for w in Wcs])
    rec_w = rec_w.astype(np.float16)
    id2 = np.stack([np.eye(128, dtype=np.float16)] * 2)
    id2[1, :, 64:] = 0.0
    idt = np.eye(128, dtype=np.float32)
    in_maps = []
    for c in range(N_CORES):
        m = {
            "X": X[c * B_LOC : (c + 1) * B_LOC],
            "W": W,
            "RW": rec_w,
            "ID2": id2,
            "IDT": idt,
        }
        if b_nonzero:
            m["BV"] = np.ascontiguousarray(b.reshape(4, 128).T)
        in_maps.append(m)
    return in_maps


def gather(results) -> np.ndarray:
    out = np.empty((B_FULL, D_OUT), dtype=np.float32)
    for c in range(N_CORES):
        o = results[c]["out"]  # (128, 4, B_LOC): unit = 128*chunk + partition
        out[c * B_LOC : (c + 1) * B_LOC] = o.transpose(2, 1, 0).reshape(B_LOC, D_OUT)
    return out


def kernel(X, W, b, Wc0, Wc1, Wc2, Wc3, Wc4, Wc5, Wc6, Wc7) -> np.ndarray:
    Wcs = [Wc0, Wc1, Wc2, Wc3, Wc4, Wc5, Wc6, Wc7]
    b_np = np.asarray(b, dtype=np.float32)
    b_nonzero = bool(np.any(b_np != 0))
    T = int(np.asarray(X).shape[1])
    nc = _get_prog(T, b_nonzero)
    in_maps = make_in_maps(X, W, b_np, Wcs, b_nonzero)
    res = run_bass_kernel_spmd(nc, in_maps, core_ids=list(range(N_CORES)))
    return gather(res.results)
